# revision 30
# baseline (speedup 1.0000x reference)
"""DLI loss kernel for Trainium2 (8 NeuronCores, SPMD data-parallel over batch).

Key algebraic fact: with scores[b,j,k] = a[b,j] + e[b,k] + fc_b (rank-1 fc),
the loss term lse_k(scores[b,j,:]) - scores[b,j,j+1] cancels a[b,j] + fc_b
exactly, so the LSTM branch and fc_w[:, :H] never affect the output:

    loss[b,j] = log(sum_{k=j+1}^{L_b-1} exp(e[b,k])) - e[b,j+1]
    e[b,k]    = encoder_output[b, ids[b,k], :] . fc_w[0, H:]

Device work per core (4 batch elements, 2 partition-groups of 2 batches):
  indirect-gather 256 turn rows -> fused dot with w_e (scalar_tensor_tensor
  with accum) -> exp -> mask -> suffix-sum via one matmul with block-diag
  upper-tri ones -> log -> per-term md = (log S - e) * mask2 -> DMA md out.
Host: shard inputs, build index/mask tables, sum the 8 cores' md tiles,
divide by count (= sum(L_b - 1), host-computable from turn_lengths alone).

Raw Bass (no Tile framework): the local walrus build caps inline sync-waits
per instruction very low, which Tile's kernel-tail drain exceeds; raw Bass
emits standalone sequencer waits instead, which have no such cap.

Scheduling notes (from CoreSim timeline):
  - consts split into a small head (idx+masks+u2) so the gathers start
    ~1us earlier while the large w-broadcast DMA overlaps them;
  - ACT LUTs for Exp/Ln are prewarmed with dummy ops during the DMA phase
    (cold table load is ~1.4us, warm evals are ~0.1us);
  - the [P,2]-offset single-gather form mis-gathers on HW (sim-only
    semantics), so two proven single-offset gathers are used.
"""

import numpy as np

_B, _S, _T = 32, 1024, 64
_E, _H = 768, 256
_NCORES = 8
_BPC = _B // _NCORES  # batches per core
_P = 128

# consts head layout (loaded first; only the gathers need it)
_C_IDX = 0  # [0, 2): gather row indices (int32 bits in f32)
_C_HEAD = 2
_C_MSK = 2  # [2, 10): masks (see _make_in_maps)
_C_U2 = 10  # [10, 138): block-diag upper-triangular ones
_C_W = 138  # [138, 906): w_e broadcast
_C_TOT = 138 + _E  # 906

_cached_nc = None


def _build_program():
    import concourse.bass as bass
    import concourse.mybir as mybir
    from contextlib import ExitStack

    f32 = mybir.dt.float32
    i32 = mybir.dt.int32
    Alu = mybir.AluOpType
    Act = mybir.ActivationFunctionType

    nc = bass.Bass()
    enc = nc.declare_dram_parameter("enc", [_BPC * _S, _E], f32, isOutput=False)
    consts = nc.declare_dram_parameter("consts", [_P, _C_TOT], f32, isOutput=False)
    out = nc.declare_dram_parameter("out", [_P, 2], f32, isOutput=True)

    with ExitStack() as ctx:
        sb = lambda name, shape: ctx.enter_context(nc.sbuf_tensor(name, shape, f32))
        ps = lambda name, shape: ctx.enter_context(nc.psum_tensor(name, shape, f32))

        c_raw = sb("c_raw", [_P, _C_TOT])
        X = sb("X", [_P, 2 * _E])
        prod0, prod1 = sb("prod0", [_P, _E]), sb("prod1", [_P, _E])
        e = sb("e", [_P, 2])
        xe = sb("xe", [_P, 2])
        mxe = sb("mxe", [_P, 2])
        ssafe = sb("ssafe", [_P, 2])
        logS = sb("logS", [_P, 2])
        diff = sb("diff", [_P, 2])
        md = sb("md", [_P, 2])
        warm = sb("warm", [_P, 2])  # scratch for ACT LUT prewarm
        s_ps = ps("s_ps", [_P, 2])

        Wt = c_raw[:, _C_W : _C_W + _E]
        ut = c_raw[:, _C_U2 : _C_U2 + _P]
        mv2 = c_raw[:, _C_MSK + 0 : _C_MSK + 2]  # valid-turn masks, both cols
        m22 = c_raw[:, _C_MSK + 2 : _C_MSK + 4]  # loss-term masks
        ma2 = c_raw[:, _C_MSK + 4 : _C_MSK + 6]  # log-safety addends
        ones_col = c_raw[:, _C_MSK + 6 : _C_MSK + 7]
        zeros = c_raw[:, _C_MSK + 7 : _C_MSK + 8]

        with (
            nc.semaphore("dma_h") as dma_h,
            nc.semaphore("dma_w") as dma_w,
            nc.semaphore("dma_o") as dma_o,
            nc.semaphore("gat0") as gat0,
            nc.semaphore("gat1") as gat1,
            nc.semaphore("dve") as dve,
            nc.semaphore("act") as act,
            nc.semaphore("pe") as pe,
            nc.Block() as block,
        ):

            @block.sync
            def _(sync):
                sync.dma_start(
                    out=c_raw[:, 0:_C_HEAD], in_=consts[:, 0:_C_HEAD]
                ).then_inc(dma_h, 16)
                sync.dma_start(
                    out=c_raw[:, _C_HEAD:_C_TOT], in_=consts[:, _C_HEAD:_C_TOT]
                ).then_inc(dma_w, 16)
                sync.wait_ge(dve, 6)  # md written
                sync.dma_start(out=out[:], in_=md[:]).then_inc(dma_o, 16)
                sync.wait_ge(dma_o, 16)  # output landed

            @block.gpsimd
            def _(gpsimd):
                gpsimd.wait_ge(dma_h, 16)  # idx cols present (head)
                # two single-offset gathers (the multi-offset [P,2] form
                # mis-gathers on HW despite simulating correctly)
                gpsimd.indirect_dma_start(
                    out=X[:, 0:_E],
                    out_offset=None,
                    in_=enc[:],
                    in_offset=bass.IndirectOffsetOnAxis(
                        ap=c_raw[:, _C_IDX : _C_IDX + 1].bitcast(i32), axis=0
                    ),
                ).then_inc(gat0, 16)
                gpsimd.indirect_dma_start(
                    out=X[:, _E : 2 * _E],
                    out_offset=None,
                    in_=enc[:],
                    in_offset=bass.IndirectOffsetOnAxis(
                        ap=c_raw[:, _C_IDX + 1 : _C_IDX + 2].bitcast(i32), axis=0
                    ),
                ).then_inc(gat1, 16)

            @block.vector
            def _(vector):
                # dve increments: 1:e0 2:e1 3:mxe 4:ssafe 5:diff 6:md
                vector.wait_ge(dma_w, 16)  # masks + w broadcast
                vector.wait_ge(gat0, 16)
                # e[:, g] = sum_f X_g[p, f] * w[f]  (fused mul + row-reduce)
                nc.vector.scalar_tensor_tensor(
                    out=prod0[:],
                    in0=X[:, 0:_E],
                    scalar=0.0,
                    in1=Wt,
                    op0=Alu.add,
                    op1=Alu.mult,
                    accum_out=e[:, 0:1],
                ).then_inc(dve, 1)
                vector.wait_ge(gat1, 16)
                nc.vector.scalar_tensor_tensor(
                    out=prod1[:],
                    in0=X[:, _E : 2 * _E],
                    scalar=0.0,
                    in1=Wt,
                    op0=Alu.add,
                    op1=Alu.mult,
                    accum_out=e[:, 1:2],
                ).then_inc(dve, 1)
                vector.wait_ge(act, 1)  # xe (both cols)
                nc.vector.tensor_mul(out=mxe[:], in0=xe[:], in1=mv2).then_inc(
                    dve, 1
                )
                vector.wait_ge(pe, 1)  # s_ps
                nc.vector.tensor_add(out=ssafe[:], in0=s_ps[:], in1=ma2).then_inc(
                    dve, 1
                )
                vector.wait_ge(act, 2)  # logS
                nc.vector.tensor_sub(out=diff[:], in0=logS[:], in1=e[:]).then_inc(
                    dve, 1
                )
                vector.wait_ge(dve, 5)  # same-engine RAW guard (deep pipeline)
                nc.vector.tensor_mul(out=md[:], in0=diff[:], in1=m22).then_inc(
                    dve, 1
                )

            @block.scalar
            def _(scalar):
                scalar.wait_ge(dma_w, 16)  # zeros col present
                # prewarm Exp/Ln LUTs during the gather phase (cold table
                # load is ~1.4us; warm evals are ~0.1us)
                nc.scalar.activation(out=warm[:, 0:1], in_=zeros, func=Act.Exp)
                nc.scalar.activation(out=warm[:, 1:2], in_=ones_col, func=Act.Ln)
                scalar.wait_ge(dve, 2)  # e (both cols)
                nc.scalar.activation(
                    out=xe[:], in_=e[:], func=Act.Exp, bias=zeros, scale=1.0
                ).then_inc(act, 1)
                scalar.wait_ge(dve, 4)  # ssafe
                nc.scalar.activation(
                    out=logS[:], in_=ssafe[:], func=Act.Ln, bias=zeros, scale=1.0
                ).then_inc(act, 1)

            @block.tensor
            def _(tensor):
                tensor.wait_ge(dma_w, 16)  # u2
                tensor.wait_ge(dve, 3)  # mxe
                # suffix sums for both groups in one matmul: S = U2.T @ mxe
                nc.tensor.matmul(
                    out=s_ps[:], lhsT=ut, rhs=mxe[:], start=True, stop=True
                ).then_inc(pe, 1)

    return nc


def _get_program():
    global _cached_nc
    if _cached_nc is None:
        _cached_nc = _build_program()
    return _cached_nc


def _make_in_maps(inputs):
    enc = np.ascontiguousarray(np.asarray(inputs["encoder_output"], dtype=np.float32))
    ids = np.asarray(inputs["his_turn_end_ids"]).astype(np.int64)
    L = np.asarray(inputs["turn_lengths"]).astype(np.int64)
    fc_w = np.asarray(inputs["fc_w"], dtype=np.float32)
    w_e = fc_w[0, _H:]

    k = np.arange(_P)
    u2v = (
        (k[:, None] // 64 == k[None, :] // 64) & (k[:, None] % 64 >= k[None, :] % 64)
    ).astype(np.float32)
    t64 = np.arange(_T)

    in_maps = []
    for c in range(_NCORES):
        sl = slice(c * _BPC, (c + 1) * _BPC)
        Lc = L[sl]
        idc = ids[sl]
        enc_c = enc[sl].reshape(_BPC * _S, _E)
        flat = (np.arange(_BPC)[:, None] * _S + idc).astype(np.int32)  # [4, 64]
        maskv = (t64[None, :] < Lc[:, None]).astype(np.float32)
        mask2 = ((t64[None, :] >= 1) & (t64[None, :] < Lc[:, None])).astype(np.float32)
        maskadd = (t64[None, :] >= Lc[:, None]).astype(np.float32)

        consts = np.zeros((_P, _C_TOT), np.float32)
        idxv = np.zeros((_P, 2), np.int32)
        idxv[:, 0] = flat[0:2].reshape(_P)
        idxv[:, 1] = flat[2:4].reshape(_P)
        consts[:, _C_IDX : _C_IDX + 2] = idxv.view(np.float32)
        for g in range(2):
            rows = slice(2 * g, 2 * g + 2)
            # msk cols: 0,1 = valid-turn mask; 2,3 = loss-term mask (1<=t<L_b);
            # 4,5 = log-safety addend (t>=L_b); 6 = ones; 7 = zeros
            consts[:, _C_MSK + g] = maskv[rows].reshape(_P)
            consts[:, _C_MSK + 2 + g] = mask2[rows].reshape(_P)
            consts[:, _C_MSK + 4 + g] = maskadd[rows].reshape(_P)
        consts[:, _C_MSK + 6] = 1.0
        consts[:, _C_U2 : _C_U2 + _P] = u2v
        consts[:, _C_W : _C_W + _E] = w_e[None, :]

        in_maps.append({"enc": enc_c, "consts": consts})
    cnt = float(np.sum(L - 1))
    return in_maps, cnt


def _run(inputs, trace=False):
    from concourse.bass_utils import run_bass_kernel_spmd

    in_maps, cnt = _make_in_maps(inputs)
    nc = _get_program()
    r = run_bass_kernel_spmd(nc, in_maps, list(range(_NCORES)), trace=trace)
    total = 0.0
    for i in range(_NCORES):
        total += float(np.asarray(r.results[i]["out"], dtype=np.float64).sum())
    return np.asarray(np.float32(total / cnt)), r


def kernel(**inputs) -> np.ndarray:
    out, _ = _run(inputs, trace=False)
    return out


# revision 32
# speedup vs baseline: 1.4066x; 1.4066x over previous
"""DLI loss kernel for Trainium2 (8 NeuronCores, SPMD data-parallel over batch).

Key algebraic fact: with scores[b,j,k] = a[b,j] + e[b,k] + fc_b (rank-1 fc),
the loss term lse_k(scores[b,j,:]) - scores[b,j,j+1] cancels a[b,j] + fc_b
exactly, so the LSTM branch and fc_w[:, :H] never affect the output:

    loss[b,j] = log(sum_{k=j+1}^{L_b-1} exp(e[b,k])) - e[b,j+1]
    e[b,k]    = encoder_output[b, ids[b,k], :] . fc_w[0, H:]

Device work per core (4 batch elements, 2 partition-groups of 2 batches):
  indirect-gather 256 turn rows -> fused dot with w_e (scalar_tensor_tensor
  with accum) -> exp -> mask -> suffix-sum via one matmul with block-diag
  upper-tri ones -> log -> per-term md = (log S - e) * mask2 -> DMA md out.
Host: shard inputs, build index/mask tables, sum the 8 cores' md tiles,
divide by count (= sum(L_b - 1), host-computable from turn_lengths alone).

Raw Bass (no Tile framework): the local walrus build caps inline sync-waits
per instruction very low, which Tile's kernel-tail drain exceeds; raw Bass
emits standalone sequencer waits instead, which have no such cap.

Scheduling notes (from CoreSim timeline):
  - consts split into a small head (idx+masks+u2) so the gathers start
    ~1us earlier while the large w-broadcast DMA overlaps them;
  - ACT LUTs for Exp/Ln are prewarmed with dummy ops during the DMA phase
    (cold table load is ~1.4us, warm evals are ~0.1us);
  - the [P,2]-offset single-gather form mis-gathers on HW (sim-only
    semantics), so two proven single-offset gathers are used.
"""

import numpy as np

_B, _S, _T = 32, 1024, 64
_E, _H = 768, 256
_NCORES = 8
_BPC = _B // _NCORES  # batches per core
_P = 128

# consts head layout (loaded first; only the gathers need it)
_C_IDX = 0  # [0, 2): gather row indices (int32 bits in f32)
_C_HEAD = 2
_C_MSK = 2  # [2, 10): masks (see _make_in_maps)
_C_U2 = 10  # [10, 138): block-diag upper-triangular ones
_C_W = 138  # [138, 906): w_e broadcast
_C_TOT = 138 + _E  # 906

_cached_nc = None


def _build_program():
    import concourse.bass as bass
    import concourse.mybir as mybir
    from contextlib import ExitStack

    f32 = mybir.dt.float32
    i32 = mybir.dt.int32
    Alu = mybir.AluOpType
    Act = mybir.ActivationFunctionType

    nc = bass.Bass()
    enc = nc.declare_dram_parameter("enc", [_BPC * _S, _E], f32, isOutput=False)
    consts = nc.declare_dram_parameter("consts", [_P, _C_TOT], f32, isOutput=False)
    out = nc.declare_dram_parameter("out", [_P, 2], f32, isOutput=True)

    with ExitStack() as ctx:
        sb = lambda name, shape: ctx.enter_context(nc.sbuf_tensor(name, shape, f32))
        ps = lambda name, shape: ctx.enter_context(nc.psum_tensor(name, shape, f32))

        c_raw = sb("c_raw", [_P, _C_TOT])
        X = sb("X", [_P, 2 * _E])
        prod0, prod1 = sb("prod0", [_P, _E]), sb("prod1", [_P, _E])
        e = sb("e", [_P, 2])
        xe = sb("xe", [_P, 2])
        logS = sb("logS", [_P, 2])
        diff = sb("diff", [_P, 2])
        md = sb("md", [_P, 2])
        warm = sb("warm", [_P, 2])  # scratch for ACT LUT prewarm
        s_ps = ps("s_ps", [_P, 2])

        Wt = c_raw[:, _C_W : _C_W + _E]
        ut = c_raw[:, _C_U2 : _C_U2 + _P]
        eb = lambda g: c_raw[:, _C_MSK + g : _C_MSK + g + 1]  # exp bias
        m22 = c_raw[:, _C_MSK + 2 : _C_MSK + 4]  # loss-term masks
        ma = lambda g: c_raw[:, _C_MSK + 4 + g : _C_MSK + 5 + g]  # log-safety
        ones_col = c_raw[:, _C_MSK + 6 : _C_MSK + 7]
        zeros = c_raw[:, _C_MSK + 7 : _C_MSK + 8]

        with (
            nc.semaphore("dma_h") as dma_h,
            nc.semaphore("dma_w") as dma_w,
            nc.semaphore("dma_o") as dma_o,
            nc.semaphore("gat0") as gat0,
            nc.semaphore("gat1") as gat1,
            nc.semaphore("dve") as dve,
            nc.semaphore("act") as act,
            nc.semaphore("pe") as pe,
            nc.Block() as block,
        ):

            @block.sync
            def _(sync):
                sync.dma_start(
                    out=c_raw[:, _C_HEAD:_C_TOT], in_=consts[:, _C_HEAD:_C_TOT]
                ).then_inc(dma_w, 16)
                sync.wait_ge(dve, 4)  # md written
                sync.dma_start(out=out[:], in_=md[:]).then_inc(dma_o, 16)
                sync.wait_ge(dma_o, 16)  # output landed

            @block.gpsimd
            def _(gpsimd):
                # idx load on the SWDGE queue (lower first-byte latency than
                # HWDGE) so the gathers can start as early as possible
                gpsimd.dma_start(
                    out=c_raw[:, 0:_C_HEAD], in_=consts[:, 0:_C_HEAD]
                ).then_inc(dma_h, 16)
                gpsimd.wait_ge(dma_h, 16)  # idx cols present (head)
                # two single-offset gathers (the multi-offset [P,2] form
                # mis-gathers on HW despite simulating correctly)
                gpsimd.indirect_dma_start(
                    out=X[:, 0:_E],
                    out_offset=None,
                    in_=enc[:],
                    in_offset=bass.IndirectOffsetOnAxis(
                        ap=c_raw[:, _C_IDX : _C_IDX + 1].bitcast(i32), axis=0
                    ),
                ).then_inc(gat0, 16)
                gpsimd.indirect_dma_start(
                    out=X[:, _E : 2 * _E],
                    out_offset=None,
                    in_=enc[:],
                    in_offset=bass.IndirectOffsetOnAxis(
                        ap=c_raw[:, _C_IDX + 1 : _C_IDX + 2].bitcast(i32), axis=0
                    ),
                ).then_inc(gat1, 16)

            @block.vector
            def _(vector):
                # dve increments: 1:e0 2:e1 3:diff 4:md
                vector.wait_ge(dma_w, 16)  # masks + w broadcast
                vector.wait_ge(gat0, 16)
                # e[:, g] = sum_f X_g[p, f] * w[f]  (fused mul + row-reduce)
                nc.vector.scalar_tensor_tensor(
                    out=prod0[:],
                    in0=X[:, 0:_E],
                    scalar=0.0,
                    in1=Wt,
                    op0=Alu.add,
                    op1=Alu.mult,
                    accum_out=e[:, 0:1],
                ).then_inc(dve, 1)
                vector.wait_ge(gat1, 16)
                nc.vector.scalar_tensor_tensor(
                    out=prod1[:],
                    in0=X[:, _E : 2 * _E],
                    scalar=0.0,
                    in1=Wt,
                    op0=Alu.add,
                    op1=Alu.mult,
                    accum_out=e[:, 1:2],
                ).then_inc(dve, 1)
                vector.wait_ge(act, 4)  # logS (both cols)
                nc.vector.tensor_sub(out=diff[:], in0=logS[:], in1=e[:]).then_inc(
                    dve, 1
                )
                vector.wait_ge(dve, 3)  # same-engine RAW guard (deep pipeline)
                nc.vector.tensor_mul(out=md[:], in0=diff[:], in1=m22).then_inc(
                    dve, 1
                )

            @block.scalar
            def _(scalar):
                scalar.wait_ge(dma_w, 16)  # bias cols present
                # prewarm Exp/Ln LUTs during the gather phase (cold table
                # load is ~1.4us; warm evals are ~0.1us)
                nc.scalar.activation(out=warm[:, 0:1], in_=zeros, func=Act.Exp)
                nc.scalar.activation(out=warm[:, 1:2], in_=ones_col, func=Act.Ln)
                # xe[:, g] = exp(e_g + bias_g): bias 0 valid / -1e30 invalid
                # folds the valid-turn mask into the exp
                scalar.wait_ge(dve, 1)  # e0
                nc.scalar.activation(
                    out=xe[:, 0:1], in_=e[:, 0:1], func=Act.Exp, bias=eb(0),
                    scale=1.0,
                ).then_inc(act, 1)
                scalar.wait_ge(dve, 2)  # e1
                nc.scalar.activation(
                    out=xe[:, 1:2], in_=e[:, 1:2], func=Act.Exp, bias=eb(1),
                    scale=1.0,
                ).then_inc(act, 1)
                # logS[:, g] = ln(S_g + maskadd_g), straight from PSUM
                scalar.wait_ge(pe, 1)
                nc.scalar.activation(
                    out=logS[:, 0:1], in_=s_ps[:, 0:1], func=Act.Ln, bias=ma(0),
                    scale=1.0,
                ).then_inc(act, 1)
                nc.scalar.activation(
                    out=logS[:, 1:2], in_=s_ps[:, 1:2], func=Act.Ln, bias=ma(1),
                    scale=1.0,
                ).then_inc(act, 1)

            @block.tensor
            def _(tensor):
                tensor.wait_ge(dma_w, 16)  # u2
                tensor.wait_ge(act, 2)  # xe (masked via exp bias)
                # suffix sums for both groups in one matmul: S = U2.T @ xe
                nc.tensor.matmul(
                    out=s_ps[:], lhsT=ut, rhs=xe[:], start=True, stop=True
                ).then_inc(pe, 1)

    return nc


def _get_program():
    global _cached_nc
    if _cached_nc is None:
        _cached_nc = _build_program()
    return _cached_nc


def _make_in_maps(inputs):
    enc = np.ascontiguousarray(np.asarray(inputs["encoder_output"], dtype=np.float32))
    ids = np.asarray(inputs["his_turn_end_ids"]).astype(np.int64)
    L = np.asarray(inputs["turn_lengths"]).astype(np.int64)
    fc_w = np.asarray(inputs["fc_w"], dtype=np.float32)
    w_e = fc_w[0, _H:]

    k = np.arange(_P)
    u2v = (
        (k[:, None] // 64 == k[None, :] // 64) & (k[:, None] % 64 >= k[None, :] % 64)
    ).astype(np.float32)
    t64 = np.arange(_T)

    in_maps = []
    for c in range(_NCORES):
        sl = slice(c * _BPC, (c + 1) * _BPC)
        Lc = L[sl]
        idc = ids[sl]
        enc_c = enc[sl].reshape(_BPC * _S, _E)
        flat = (np.arange(_BPC)[:, None] * _S + idc).astype(np.int32)  # [4, 64]
        maskv = (t64[None, :] < Lc[:, None]).astype(np.float32)
        mask2 = ((t64[None, :] >= 1) & (t64[None, :] < Lc[:, None])).astype(np.float32)
        maskadd = (t64[None, :] >= Lc[:, None]).astype(np.float32)

        consts = np.zeros((_P, _C_TOT), np.float32)
        idxv = np.zeros((_P, 2), np.int32)
        idxv[:, 0] = flat[0:2].reshape(_P)
        idxv[:, 1] = flat[2:4].reshape(_P)
        consts[:, _C_IDX : _C_IDX + 2] = idxv.view(np.float32)
        for g in range(2):
            rows = slice(2 * g, 2 * g + 2)
            # msk cols: 0,1 = exp bias (0 valid, -1e30 masks t>=L_b);
            # 2,3 = loss-term mask (1<=t<L_b); 4,5 = log-safety addend
            # (t>=L_b); 6 = ones; 7 = zeros
            consts[:, _C_MSK + g] = (maskv[rows].reshape(_P) - 1.0) * 1e30
            consts[:, _C_MSK + 2 + g] = mask2[rows].reshape(_P)
            consts[:, _C_MSK + 4 + g] = maskadd[rows].reshape(_P)
        consts[:, _C_MSK + 6] = 1.0
        consts[:, _C_U2 : _C_U2 + _P] = u2v
        consts[:, _C_W : _C_W + _E] = w_e[None, :]

        in_maps.append({"enc": enc_c, "consts": consts})
    cnt = float(np.sum(L - 1))
    return in_maps, cnt


def _run(inputs, trace=False):
    from concourse.bass_utils import run_bass_kernel_spmd

    in_maps, cnt = _make_in_maps(inputs)
    nc = _get_program()
    r = run_bass_kernel_spmd(nc, in_maps, list(range(_NCORES)), trace=trace)
    total = 0.0
    for i in range(_NCORES):
        total += float(np.asarray(r.results[i]["out"], dtype=np.float64).sum())
    return np.asarray(np.float32(total / cnt)), r


def kernel(**inputs) -> np.ndarray:
    out, _ = _run(inputs, trace=False)
    return out
